# revision 6
# baseline (speedup 1.0000x reference)
"""Trainium2 Bass kernel for DeepSupervisionLoss (focal + boundary-weighted dice,
4 heads, deep supervision). Pure data-parallel over the batch dim across 8 cores;
each core reduces its shard to 16 partial scalars; host combines.

Math (per element, t binary, z = x*(2t-1)):
  bce  = softplus(-z) = ln(1 + exp(-z))
  pt=v = sigmoid(z)   = exp(-bce)
  u^2.5 = exp(-2.5*(z + bce))           (u = 1 - pt)
  focal_elem = 0.25 * u^2.5 * bce
  boundary b = maxpool3x3(t) + maxpool3x3(1-t) - 1   (in {0,1})
  w = 1 + 7b = 7c - 6 with c = Dx + Dn; wt = w*t
  I  = <v, wt>;  VW = <v, w>;  P = 2I + (S_w - S_wt) - VW;  T = S_wt
  dice = (2I+1)/(P+T+1);  head = 0.3*focal_mean + 0.7*(1-dice)
Sums <.,.> are computed on the PE as diagonals of accumulated A^T B in PSUM,
with the two dice dots batched over all 4 heads per matmul (moving = v4 planes).
Cross-partition row shifts for the 3x3 pools are SBUF->SBUF DMA copies (the
PE-shift + PSUM-read path costs ~2.3us per DVE max op; SBUF bf16 costs 0.4us).
"""
import sys

import numpy as np

for _p in ("/opt/trn_rl_repo",):
    if _p not in sys.path:
        sys.path.insert(0, _p)

import ml_dtypes  # noqa: E402

import concourse.bacc as bacc  # noqa: E402
import concourse.mybir as mybir  # noqa: E402
from concourse import tile  # noqa: E402
from concourse.alu_op_type import AluOpType  # noqa: E402

F32 = mybir.dt.float32
BF16 = mybir.dt.bfloat16
AF = mybir.ActivationFunctionType

N_CORES = 8
N_IMG_TOTAL = 32
H = W = 512
P = 128              # partitions
RB = 4               # rows per partition
FD = RB * W          # 2048 free elems per image tile
NCH = 16             # 128-col chunks per tile
PRED_NAMES = ("main_pred", "ds1", "ds2", "ds3")


def build_nc(n_img):
    nc = bacc.Bacc("TRN2", target_bir_lowering=False, debug=False)

    xs = [nc.declare_dram_parameter(nm, [n_img, H, W], F32, isOutput=False)
          for nm in PRED_NAMES]
    tg = nc.declare_dram_parameter("target", [n_img, H, W], F32, isOutput=False)
    ident_d = nc.declare_dram_parameter("ident", [P, P], F32, isOutput=False)
    ones_d = nc.declare_dram_parameter("onescol", [P, 1], F32, isOutput=False)
    out_d = nc.declare_dram_parameter("out", [1, 16], F32, isOutput=True)

    def img_view(dram, i):
        # [512, 512] image -> [128, 2048]; partition p holds rows 4p..4p+3
        return dram.ap()[i].rearrange("(p a) w -> p (a w)", p=P)

    with tile.TileContext(nc) as tc:
        with (
            tc.tile_pool(name="consts", bufs=1) as cp,
            tc.tile_pool(name="tgt", bufs=1) as tp_,
            tc.tile_pool(name="tgt2", bufs=2) as tp2,
            tc.tile_pool(name="pred", bufs=2) as pp,
            tc.tile_pool(name="xin", bufs=3) as xp,
            tc.tile_pool(name="v4p", bufs=2) as vp,
            tc.tile_pool(name="psacc", bufs=1, space="PSUM") as pa,
            tc.tile_pool(name="pssh", bufs=1, space="PSUM") as ps,
        ):
            ident = cp.tile([P, P], F32)
            onescol = cp.tile([P, 1], F32)
            nc.sync.dma_start(out=ident[:], in_=ident_d.ap())
            nc.sync.dma_start(out=onescol[:], in_=ones_d.ap())

            swt_cols = cp.tile([P, n_img], F32)
            sw_cols = cp.tile([P, n_img], F32)
            acc16 = cp.tile([P, 16], F32)
            nc.vector.memset(acc16[:], 0.0)

            # PSUM accumulators: diag(A^T B) accumulation targets.
            # Pre-zeroed; all matmuls accumulate (start=False) so Tile's
            # PE reordering cannot race a start=True clear against earlier
            # contributions (order of pure accumulates is commutative).
            accI = pa.tile([P, 4, P], F32)    # per pred: <v, wt>  (1 bank)
            accV = pa.tile([P, 4, P], F32)    # per pred: <v, w>   (1 bank)
            accF = pa.tile([P, 4, P], F32)    # per pred: focal    (1 bank)
            nc.vector.memset(accI[:], 0.0)
            nc.vector.memset(accV[:], 0.0)
            nc.vector.memset(accF[:], 0.0)

            # persistent cross-partition shift tiles (2 sets, by image parity).
            # DMA writes rows 1..127 (or 0..126); the boundary row stays 0
            # forever (zero-fill is exact for max over binary data).
            shtiles = []
            for par in range(2):
                st = {}
                for nm in ("x3", "x0", "n3", "n0"):
                    t_ = cp.tile([P, W], BF16, name=f"sh{par}{nm}")
                    nc.vector.memset(t_[:], 0.0)
                    st[nm] = t_
                shtiles.append(st)

            for img in range(n_img):
                sh = shtiles[img % 2]
                # ---------------- target pipeline ----------------
                t_f32 = tp2.tile([P, FD], F32, name="t_f32")
                nc.sync.dma_start(out=t_f32[:], in_=img_view(tg, img))

                msign = tp_.tile([P, FD], BF16, name="msign")     # 2t-1
                nc.vector.tensor_scalar(
                    out=msign[:], in0=t_f32[:],
                    scalar1=2.0, scalar2=-1.0, op0=AluOpType.mult, op1=AluOpType.add)
                tb = tp_.tile([P, RB, W], BF16, name="tb")        # t  in bf16
                nc.vector.tensor_scalar(
                    out=tb[:], in0=msign[:].rearrange("p (a w) -> p a w", a=RB),
                    scalar1=0.5, scalar2=0.5, op0=AluOpType.mult, op1=AluOpType.add)
                tp = tp_.tile([P, RB, W], BF16, name="tp")        # 1-t (Pool)
                nc.gpsimd.tensor_scalar(
                    out=tp[:], in0=msign[:].rearrange("p (a w) -> p a w", a=RB),
                    scalar1=-0.5, scalar2=0.5, op0=AluOpType.mult, op1=AluOpType.add)

                # horizontal 3-tap max, sliced (no padded buffers):
                # Ax[c] = max(t[c-1], t[c+1]) interior, copies at borders;
                # hx = max(Ax, t).
                Ax = tp_.tile([P, RB, W], BF16, name="Ax")
                hx = tp_.tile([P, RB, W], BF16, name="hx")
                nc.vector.tensor_copy(out=Ax[:, :, 0:1], in_=tb[:, :, 1:2])
                nc.vector.tensor_copy(out=Ax[:, :, W - 1:W], in_=tb[:, :, W - 2:W - 1])
                nc.vector.tensor_tensor(out=Ax[:, :, 1:W - 1], in0=tb[:, :, 0:W - 2],
                                        in1=tb[:, :, 2:W], op=AluOpType.max)
                nc.vector.tensor_tensor(out=hx[:], in0=Ax[:], in1=tb[:], op=AluOpType.max)
                # 1-t tree horizontal on Pool
                An = tp_.tile([P, RB, W], BF16, name="An")
                hn = tp_.tile([P, RB, W], BF16, name="hn")
                nc.vector.tensor_copy(out=An[:, :, 0:1], in_=tp[:, :, 1:2])
                nc.vector.tensor_copy(out=An[:, :, W - 1:W], in_=tp[:, :, W - 2:W - 1])
                nc.vector.tensor_tensor(out=An[:, :, 1:W - 1], in0=tp[:, :, 0:W - 2],
                                        in1=tp[:, :, 2:W], op=AluOpType.max)
                nc.vector.tensor_tensor(out=hn[:], in0=An[:], in1=tp[:], op=AluOpType.max)

                # cross-partition rows via SBUF->SBUF DMA shifts (SWDGE:
                # HWDGE descriptors lack wait slots for the WAR+RAW deps here)
                nc.gpsimd.dma_start(out=sh["x3"][1:P, :], in_=hx[0:P - 1, 3, :])
                nc.gpsimd.dma_start(out=sh["x0"][0:P - 1, :], in_=hx[1:P, 0, :])
                nc.gpsimd.dma_start(out=sh["n3"][1:P, :], in_=hn[0:P - 1, 3, :])
                nc.gpsimd.dma_start(out=sh["n0"][0:P - 1, :], in_=hn[1:P, 0, :])

                # vertical 3-tap max within/across partitions
                Dx = tp_.tile([P, RB, W], BF16, name="Dx")
                Dn = tp_.tile([P, RB, W], BF16, name="Dn")
                for (hsrc, dst, sh3, sh0) in ((hx, Dx, sh["x3"], sh["x0"]),
                                              (hn, Dn, sh["n3"], sh["n0"])):
                    m12 = tp_.tile([P, W], BF16, name="m12")
                    nc.vector.tensor_tensor(out=m12[:], in0=hsrc[:, 1, :],
                                            in1=hsrc[:, 2, :], op=AluOpType.max)
                    nc.vector.tensor_tensor(out=dst[:, 1, :], in0=hsrc[:, 0, :],
                                            in1=m12[:], op=AluOpType.max)
                    nc.vector.tensor_tensor(out=dst[:, 2, :], in0=m12[:],
                                            in1=hsrc[:, 3, :], op=AluOpType.max)
                    v0a = tp_.tile([P, W], BF16, name="v0a")
                    nc.vector.tensor_tensor(out=v0a[:], in0=hsrc[:, 0, :],
                                            in1=hsrc[:, 1, :], op=AluOpType.max)
                    nc.vector.tensor_tensor(out=dst[:, 0, :], in0=v0a[:],
                                            in1=sh3[:], op=AluOpType.max)
                    v3a = tp_.tile([P, W], BF16, name="v3a")
                    nc.vector.tensor_tensor(out=v3a[:], in0=hsrc[:, 2, :],
                                            in1=hsrc[:, 3, :], op=AluOpType.max)
                    nc.vector.tensor_tensor(out=dst[:, 3, :], in0=v3a[:],
                                            in1=sh0[:], op=AluOpType.max)

                # c = Dx + Dn in {1,2};  w = 7c - 6 in {1,8};  wt = w*t
                cc = tp_.tile([P, FD], BF16, name="cc")
                nc.vector.tensor_tensor(
                    out=cc[:], in0=Dx[:].rearrange("p a w -> p (a w)"),
                    in1=Dn[:].rearrange("p a w -> p (a w)"), op=AluOpType.add)
                WW = tp_.tile([P, 2, FD], BF16, name="WW")
                nc.vector.tensor_scalar(
                    out=WW[:, 1, :], in0=cc[:], scalar1=7.0, scalar2=-6.0,
                    op0=AluOpType.mult, op1=AluOpType.add,
                    accum_out=sw_cols[:, img:img + 1])
                nc.vector.scalar_tensor_tensor(
                    out=WW[:, 0, :], in0=WW[:, 1, :], scalar=1.0,
                    in1=tb[:].rearrange("p a w -> p (a w)"),
                    op0=AluOpType.mult, op1=AluOpType.mult,
                    accum_out=swt_cols[:, img:img + 1])

                # ---------------- pred pipeline (4 heads) ----------------
                v4 = vp.tile([P, 4, FD], BF16, name="v4")
                for k in range(4):
                    x_t = xp.tile([P, FD], BF16, name="x_t")
                    # SWDGE (casts f32->bf16 in flight; also HWDGE descriptors
                    # have too few wait slots for this load's WAR deps).
                    nc.gpsimd.dma_start(out=x_t[:], in_=img_view(xs[k], img))

                    z_t = pp.tile([P, FD], BF16, name="z_t")
                    nc.vector.tensor_tensor(out=z_t[:], in0=x_t[:], in1=msign[:],
                                            op=AluOpType.mult)
                    e_t = pp.tile([P, FD], BF16, name="e_t")
                    nc.scalar.activation(e_t[:], z_t[:], AF.Exp, scale=-1.0)
                    bce_t = pp.tile([P, FD], BF16, name="bce_t")
                    nc.scalar.activation(bce_t[:], e_t[:], AF.Ln, bias=1.0)
                    nc.scalar.activation(v4[:, k, :], bce_t[:], AF.Exp, scale=-1.0)
                    q_t = pp.tile([P, FD], BF16, name="q_t")
                    nc.vector.tensor_tensor(out=q_t[:], in0=z_t[:], in1=bce_t[:],
                                            op=AluOpType.add)
                    u25_t = pp.tile([P, FD], BF16, name="u25_t")
                    nc.scalar.activation(u25_t[:], q_t[:], AF.Exp, scale=-2.5)

                    last = img == n_img - 1
                    for c in range(NCH):
                        cs = slice(c * P, (c + 1) * P)
                        nc.tensor.matmul(
                            accF[:, k, :],
                            u25_t[:, cs], bce_t[:, cs],
                            start=False, stop=(last and c == NCH - 1),
                            skip_group_check=True)

                last = img == n_img - 1
                for c in range(NCH):
                    cs = slice(c * P, (c + 1) * P)
                    nc.tensor.matmul(
                        accI[:],
                        WW[:, 0, cs], v4[:, :, cs],
                        start=False, stop=(last and c == NCH - 1),
                        skip_group_check=True)
                    nc.tensor.matmul(
                        accV[:],
                        WW[:, 1, cs], v4[:, :, cs],
                        start=False, stop=(last and c == NCH - 1),
                        skip_group_check=True)

            # ---------------- final reduction ----------------
            nc.vector.tensor_reduce(out=acc16[:, 12:13], in_=swt_cols[:],
                                    axis=mybir.AxisListType.X, op=AluOpType.add)
            nc.vector.tensor_reduce(out=acc16[:, 13:14], in_=sw_cols[:],
                                    axis=mybir.AxisListType.X, op=AluOpType.add)
            dscr = cp.tile([P, P], F32)
            for j in range(4):
                nc.vector.scalar_tensor_tensor(
                    out=dscr[:], in0=accI[:, j, :], scalar=1.0, in1=ident[:],
                    op0=AluOpType.mult, op1=AluOpType.mult,
                    accum_out=acc16[:, j:j + 1])
                nc.vector.scalar_tensor_tensor(
                    out=dscr[:], in0=accV[:, j, :], scalar=1.0, in1=ident[:],
                    op0=AluOpType.mult, op1=AluOpType.mult,
                    accum_out=acc16[:, 4 + j:5 + j])
                nc.vector.scalar_tensor_tensor(
                    out=dscr[:], in0=accF[:, j, :], scalar=1.0, in1=ident[:],
                    op0=AluOpType.mult, op1=AluOpType.mult,
                    accum_out=acc16[:, 8 + j:9 + j])

            fin = ps.tile([1, 16], F32, name="fin")
            nc.tensor.matmul(fin[:], onescol[:], acc16[:], start=True, stop=True)
            out_sb = cp.tile([1, 16], F32)
            nc.vector.tensor_copy(out=out_sb[:], in_=fin[:])
            nc.sync.dma_start(out=out_d.ap(), in_=out_sb[:])

    _pin_act_table_set(nc)
    nc.finalize()
    return nc


def _pin_act_table_set(nc, set_name="natural_log_exp_and_others"):
    """All our ACT funcs (Exp, Ln) live in one table set, but the stock
    insertion pass alternates exp_and_others/natural_log per instruction
    (~31 reloads x ~1.3us on the critical ScalarE). Pin every load to the
    combined set and drop duplicates."""
    orig = nc.insert_act_table_loads

    def patched():
        orig()
        from concourse.hw_specs import get_activation_tables
        names = list(get_activation_tables(nc.m.arch).keys())
        cid = names.index(set_name)
        for fn in nc.m.functions:
            for blk in fn.blocks:
                seen = False
                kept = []
                for ins in blk.instructions:
                    if isinstance(ins, mybir.InstLoadActFuncSet):
                        if seen:
                            continue
                        ins.act_func_set_id = cid
                        seen = True
                    kept.append(ins)
                if len(kept) != len(blk.instructions):
                    blk.instructions[:] = kept

    nc.insert_act_table_loads = patched


def _consts():
    ident = np.eye(P, dtype=np.float32)
    ones = np.ones((P, 1), dtype=np.float32)
    return {"ident": ident, "onescol": ones}


_NC_CACHE = {}


def _get_nc(n_img):
    if n_img not in _NC_CACHE:
        _NC_CACHE[n_img] = build_nc(n_img)
    return _NC_CACHE[n_img]


def combine_partials(outs, n_total_elems):
    """outs: list of [1,16] f32 per core -> final scalar (float64 host math)."""
    s = np.zeros(16, dtype=np.float64)
    for o in outs:
        s += np.asarray(o, dtype=np.float64).reshape(16)
    I = [s[k] for k in range(4)]
    VW = [s[4 + k] for k in range(4)]
    F = [s[8 + k] for k in range(4)]
    S_wt, S_w = s[12], s[13]
    total = 0.0
    for k, c in enumerate((1.0, 0.4, 0.2, 0.1)):
        f = 0.25 * F[k] / n_total_elems
        Pk = 2.0 * I[k] + (S_w - S_wt) - VW[k]
        dice = (2.0 * I[k] + 1.0) / (Pk + S_wt + 1.0)
        total += c * (0.3 * f + 0.7 * (1.0 - dice))
    return np.float32(total)


def kernel(main_pred, ds1, ds2, ds3, target, _trace=False):
    from concourse.bass_utils import run_bass_kernel_spmd

    n_img = N_IMG_TOTAL // N_CORES
    nc = _get_nc(n_img)
    consts = _consts()
    preds = {"main_pred": main_pred, "ds1": ds1, "ds2": ds2, "ds3": ds3}
    in_maps = []
    for core in range(N_CORES):
        sl = slice(core * n_img, (core + 1) * n_img)
        m = {nm: np.ascontiguousarray(
                np.asarray(v).reshape(N_IMG_TOTAL, H, W)[sl]).astype(np.float32)
             for nm, v in preds.items()}
        m["target"] = np.ascontiguousarray(
            np.asarray(target).reshape(N_IMG_TOTAL, H, W)[sl]).astype(np.float32)
        m.update(consts)
        in_maps.append(m)

    res = run_bass_kernel_spmd(nc, in_maps, list(range(N_CORES)), trace=_trace)
    outs = [r["out"] for r in res.results]
    total = combine_partials(outs, N_IMG_TOTAL * H * W)
    if _trace:
        kernel._last_result = res
    return np.asarray(total, dtype=np.float32)
